# revision 14
# baseline (speedup 1.0000x reference)
"""Trainium2 Bass kernel for nn_MemoryModule (sparse_attention).

Reference computation (shapes hardcoded):
  B=2, T=4, Ck=64, Cv=256, H=32, W=64;  M=T*H*W=8192, N=H*W=2048
  mk   = memory_keys_low.transpose(0,2,1,3,4).reshape(B, Ck, M)
  qk   = query_key_low.reshape(B, Ck, N)
  attn = softmax_m(mk^T qk)            # [B, M, N]
  mem  = mv @ attn                     # [B, Cv, N], mv = [B, Cv, M]
  qv   = avgpool2x2(query_value)       # bilinear downsize == 2x2 avgpool here
  out  = concat([qv, mem], axis=1)     # [B, 512, 32, 64]

Sharding: 8 cores = 2 batches x 4 query-chunks of 512 positions each; the
softmax axis (m) stays local per core.

Numerics: logits span +-265 for these inputs, so softmax needs a per-query
shift. The kernel runs flash-attention style over 4 quarters of the memory
axis: each quarter uses a running per-column max (of that and all previous
quarters), so exp() never overflows and expattn fits fp16. The shift is
applied inside matmul1 itself: row 64 of the stationary is ones and row 64
of the pass-B moving operand is -shift. Quarter maxima are computed by a
transposed fp16 matmul + DVE free-axis max-reduce, interleaved into the
previous quarter's pipeline (DVE is otherwise idle there). At quarter
boundaries the PSUM accumulators are rescaled by exp(old-new) with shift
values rounded to fp16 first so the rescale factor exactly matches the
shift the matmul applied.

matmul1 is an fp16 hi/lo split of both operands packed into K=128 twice
(pass A: [hi;1;lo'] x [qk_hi;0;qk_hi'], pass B: same x [qk_lo;-c;qk_lo']),
giving near-fp32 logits at full PE rate. The ones column appended to mv
yields the softmax denominator through the same matmul accumulation.
"""

import os
import sys

sys.path.insert(0, "/opt/trn_rl_repo")

import numpy as np
import ml_dtypes

import concourse.bass as bass
import concourse.tile as tile
import concourse.mybir as mybir
from concourse import bacc
from concourse.bass_utils import run_bass_kernel_spmd
from concourse.masks import make_identity

B, T, CK, CV, H, W = 2, 4, 64, 256, 32, 64
M = T * H * W            # 8192 memory positions
N = H * W                # 2048 query positions
NCHUNK = 512             # query positions per core
NCORES = 8
MT = M // 128            # 64 m-tiles
PART_SIZES = [8, 12, 16, 16, 12]   # m-tiles per flash part (sum = 64)
NPART = len(PART_SIZES)
PART_STARTS = [sum(PART_SIZES[:i]) for i in range(NPART)]
QH, QW = 64, 128         # query_value spatial dims (2x the output)

F32 = mybir.dt.float32
F16 = mybir.dt.float16
AX = mybir.AxisListType
OP = mybir.AluOpType
ACTF = mybir.ActivationFunctionType

_cached = {}


def _build_program():
    nc = bacc.Bacc("TRN2", target_bir_lowering=False, debug=False,
                   num_devices=NCORES)

    mk = nc.dram_tensor("mk", [128, M], F16, kind="ExternalInput").ap()
    rhsA = nc.dram_tensor("rhsA", [128, NCHUNK], F16, kind="ExternalInput").ap()
    rhsB = nc.dram_tensor("rhsB", [128, NCHUNK], F16, kind="ExternalInput").ap()
    mvp = nc.dram_tensor("mvp", [MT, 128, 257], F16, kind="ExternalInput").ap()
    qv = nc.dram_tensor("qv", [2, 128, 16, QW], F32, kind="ExternalInput").ap()

    mout = nc.dram_tensor("mout", [4, 128, 256], F32, kind="ExternalOutput").ap()
    qvout = nc.dram_tensor("qvout", [2, 128, 8, 64], F32, kind="ExternalOutput").ap()

    with tile.TileContext(nc) as tc:
        with (
            tc.tile_pool(name="big", bufs=1) as big,
            tc.tile_pool(name="mvp", bufs=8) as mvp_pool,
            tc.tile_pool(name="ea", bufs=5) as ea_pool,
            tc.tile_pool(name="cmp", bufs=2) as cm_pool,
            tc.tile_pool(name="qvp", bufs=2) as qv_pool,
            tc.tile_pool(name="outp", bufs=2) as out_pool,
            tc.tile_pool(name="pst", bufs=2, space=bass.MemorySpace.PSUM) as pst_pool,
            tc.tile_pool(name="ps1", bufs=2, space=bass.MemorySpace.PSUM) as ps1_pool,
            tc.tile_pool(name="acc", bufs=1, space=bass.MemorySpace.PSUM) as acc_pool,
        ):
            # ---- stationary inputs, priority order: what the first
            # colmax chunks and first mm1 tiles need arrives first ----
            ra_t = big.tile([128, NCHUNK], F16, tag="ra")
            nc.gpsimd.dma_start(ra_t[:], rhsA[:])
            mk_t = big.tile([128, M], F16, tag="mk")
            nc.gpsimd.dma_start(mk_t[0:64, 0:1024], mk[0:64, 0:1024])
            nc.gpsimd.dma_start(mk_t[64:128, 0:1024], mk[64:128, 0:1024])
            rb_ts = []
            for h in range(2):
                rb = big.tile([128, NCHUNK], F16, tag=f"rb{h}", name=f"rb{h}")
                nc.gpsimd.dma_start(rb[:], rhsB[:])
                rb_ts.append(rb)
            nc.gpsimd.dma_start(mk_t[0:64, 1024:2560], mk[0:64, 1024:2560])
            nc.gpsimd.dma_start(mk_t[64:128, 1024:2560], mk[64:128, 1024:2560])
            nc.gpsimd.dma_start(mk_t[0:64, 2560:M], mk[0:64, 2560:M])
            nc.gpsimd.dma_start(mk_t[64:128, 2560:M], mk[64:128, 2560:M])
            ident = big.tile([128, 128], F16, tag="ident")
            make_identity(nc, ident[:])

            accs = [acc_pool.tile([128, 257], F32, tag=f"acc{j}",
                                  name=f"acc{j}") for j in range(4)]
            # running shift (fp16-rounded), per n-subtile
            cfs = [None] * 4

            def part_chunks(part):
                c0 = PART_STARTS[part] // 4
                c1 = c0 + PART_SIZES[part] // 4
                return list(range(c0, c1))

            def emit_colmax_chunk(i, c, cm4):
                """One MM_T chunk + its max-reduce into cm4 column."""
                pst = pst_pool.tile([128, 512], F32, tag="pst", name="pst")
                nc.tensor.matmul(
                    pst[:],
                    ra_t[0:64, i * 128:(i + 1) * 128],
                    mk_t[0:64, c * 512:(c + 1) * 512],
                    start=True, stop=True,
                )
                nc.vector.tensor_reduce(
                    cm4[:, c % (cm4.shape[1]):c % (cm4.shape[1]) + 1],
                    pst[:], axis=AX.X, op=OP.max)

            def colmax_part(part):
                """Emit MM_T + reduces for part's m-range; returns c tiles."""
                chunks = part_chunks(part)
                cparts = []
                for i in range(4):
                    cm4 = cm_pool.tile([128, len(chunks)], F32, tag=f"cm4_{i}",
                                       name=f"cm4_{i}")
                    for ci, c in enumerate(chunks):
                        pst = pst_pool.tile([128, 512], F32, tag="pst",
                                            name="pst")
                        nc.tensor.matmul(
                            pst[:],
                            ra_t[0:64, i * 128:(i + 1) * 128],
                            mk_t[0:64, c * 512:(c + 1) * 512],
                            start=True, stop=True,
                        )
                        nc.vector.tensor_reduce(
                            cm4[:, ci:ci + 1], pst[:], axis=AX.X, op=OP.max)
                    cp = cm_pool.tile([128, 1], F16, tag=f"cp{i}",
                                      name=f"cp{i}")
                    # fp16 rounding here defines the exact shift value used
                    nc.vector.tensor_reduce(cp[:], cm4[:], axis=AX.X, op=OP.max)
                    cparts.append(cp)
                return cparts

            def write_shift_row(rb, shifts):
                """rb[64, :] = -shifts (via PE transpose of [128,1]->[1,128])."""
                for i in range(4):
                    cmT = pst_pool.tile([1, 128], F16, tag="pst", name="cmT")
                    nc.tensor.transpose(cmT[:], shifts[i][:], ident[:])
                    nc.vector.tensor_scalar_mul(
                        rb[64:65, i * 128:(i + 1) * 128], cmT[0:1, :], -1.0)

            # ---- phase A: colmax of quarter 0 ----
            with nc.named_scope("colmax0"):
                cfs = colmax_part(0)
                write_shift_row(rb_ts[0], cfs)

            pending = []
            DEPTH = 2

            def flush_one():
                ea_p, mv_p, kp = pending.pop(0)
                for j in range(4):
                    nc.tensor.matmul(
                        accs[j][:],
                        ea_p[:, j * 128:(j + 1) * 128],
                        mv_p[:],
                        start=(kp == 0),
                        stop=(kp == MT - 1),
                        skip_group_check=True,
                    )

            for part in range(NPART):
                rb = rb_ts[part % 2]
                psize = PART_SIZES[part]
                pstart = PART_STARTS[part]
                # schedule of next-part colmax work: (n_tile, chunk) pairs
                # spread evenly over this part's iterations
                if part < NPART - 1:
                    nxt_chunks = part_chunks(part + 1)
                    work = [(i, c) for i in range(4) for c in nxt_chunks]
                    next_cm4 = [cm_pool.tile([128, len(nxt_chunks)], F32,
                                             tag=f"cm4_{i}", name=f"cm4_{i}")
                                for i in range(4)]
                else:
                    work = []
                    next_cm4 = None
                with nc.named_scope(f"part{part}"):
                    for kk in range(psize):
                        k = pstart + kk
                        mv_t = mvp_pool.tile([128, 257], F16, tag="mvt")
                        nc.sync.dma_start(mv_t[:], mvp[k])

                        ps1 = ps1_pool.tile([128, NCHUNK], F32, tag="ps1")
                        lhs = mk_t[:, k * 128:(k + 1) * 128]
                        nc.tensor.matmul(ps1[:], lhs, ra_t[:],
                                         start=True, stop=False)
                        nc.tensor.matmul(ps1[:], lhs, rb[:],
                                         start=False, stop=True)

                        ea = ea_pool.tile([128, NCHUNK], F16, tag="ea")
                        nc.scalar.activation(ea[:], ps1[:], ACTF.Exp)
                        pending.append((ea, mv_t, k))
                        if len(pending) > DEPTH:
                            flush_one()

                        # interleave next part's colmax work evenly,
                        # finishing a few iterations before the part ends
                        if work:
                            total = 4 * len(nxt_chunks)
                            budget = max(1, psize - 3)
                            quota = min(total, total * (kk + 1) // budget)
                            while len(work) > total - quota:
                                i, c = work.pop(0)
                                ci = c - nxt_chunks[0]
                                pst = pst_pool.tile([128, 512], F32,
                                                    tag="pst", name="pst")
                                nc.tensor.matmul(
                                    pst[:],
                                    ra_t[0:64, i * 128:(i + 1) * 128],
                                    mk_t[0:64, c * 512:(c + 1) * 512],
                                    start=True, stop=True,
                                )
                                nc.vector.tensor_reduce(
                                    next_cm4[i][:, ci:ci + 1], pst[:],
                                    axis=AX.X, op=OP.max)
                            # all colmax work done -> prep next part's
                            # shift row now so its mm1 is not blocked on
                            # the boundary
                            if not work:
                                fs = []
                                new_cfs = []
                                with nc.named_scope(f"prep{part}"):
                                    for i in range(4):
                                        cp = cm_pool.tile(
                                            [128, 1], F16, tag=f"cp{i}",
                                            name=f"cp{i}")
                                        nc.vector.tensor_reduce(
                                            cp[:], next_cm4[i][:],
                                            axis=AX.X, op=OP.max)
                                        cfn = cm_pool.tile(
                                            [128, 1], F16, tag=f"cfn{i}",
                                            name=f"cfn{i}")
                                        nc.vector.tensor_tensor(
                                            cfn[:], cfs[i][:], cp[:],
                                            op=OP.max)
                                        d = cm_pool.tile(
                                            [128, 1], F32, tag=f"d{i}",
                                            name=f"d{i}")
                                        nc.vector.tensor_tensor(
                                            d[:], cfs[i][:], cfn[:],
                                            op=OP.subtract)
                                        f = cm_pool.tile(
                                            [128, 1], F32, tag=f"f{i}",
                                            name=f"f{i}")
                                        nc.scalar.activation(
                                            f[:], d[:], ACTF.Exp)
                                        fs.append(f)
                                        new_cfs.append(cfn)
                                    write_shift_row(
                                        rb_ts[(part + 1) % 2], new_cfs)

                    # ---- part boundary: flush mm2 then rescale accs ----
                    while pending:
                        flush_one()

                    if part == NPART - 1:
                        break
                    with nc.named_scope(f"boundary{part}"):
                        for i in range(4):
                            nc.vector.tensor_scalar_mul(
                                accs[i][:], accs[i][:], fs[i][:])
                        cfs = new_cfs

                # qv pooling emitted after part 1 so its DVE/DMA work lands
                # in the (DVE-light) later parts
                if part == 1:
                    for p in range(2):
                        qt = qv_pool.tile([128, 16, QW], F32, tag="qt")
                        nc.gpsimd.dma_start(qt[:], qv[p])
                        t1 = qv_pool.tile([128, 16, 64], F32, tag="t1")
                        nc.vector.tensor_add(t1[:], qt[:, :, 0:QW:2],
                                             qt[:, :, 1:QW:2])
                        t2 = qv_pool.tile([128, 8, 64], F32, tag="t2")
                        nc.vector.tensor_add(t2[:], t1[:, 0:16:2, :],
                                             t1[:, 1:16:2, :])
                        qo = qv_pool.tile([128, 8, 64], F32, tag="qo")
                        nc.vector.tensor_scalar_mul(qo[:], t2[:], 0.25)
                        nc.gpsimd.dma_start(qvout[p], qo[:])

            # ---- normalize and write out ----
            with nc.named_scope("norm"):
                for j in range(4):
                    rec = out_pool.tile([128, 1], F32, tag="rec")
                    nc.vector.reciprocal(rec[:], accs[j][:, 256:257])
                    ot = out_pool.tile([128, 256], F32, tag="ot")
                    nc.vector.tensor_scalar_mul(ot[:], accs[j][:, 0:256], rec[:])
                    nc.gpsimd.dma_start(mout[j], ot[:])

    nc.compile()
    return nc


def _prep_inputs(query_value, memory_keys_low, memory_values_low, query_key_low):
    """Host-side shard + layout prep. Returns in_maps for the 8 cores."""
    f16 = np.float16
    in_maps = []
    for b in range(B):
        mk = np.ascontiguousarray(
            memory_keys_low[b].transpose(1, 0, 2, 3).reshape(CK, M))
        qk = np.ascontiguousarray(query_key_low[b].reshape(CK, N))

        mk_hi = mk.astype(f16)
        mk_lo = (mk - mk_hi.astype(np.float32)).astype(f16)
        qk_hi = qk.astype(f16)
        qk_lo = (qk - qk_hi.astype(np.float32)).astype(f16)

        # One lo-channel is sacrificed for the ones/-shift row; permute the
        # channel whose lo x qk product is smallest into the last slot.
        d = (np.abs(mk_lo.astype(np.float32)).max(axis=1)
             * np.abs(qk).max(axis=1))
        c_drop = int(np.argmin(d))
        perm = list(range(CK))
        perm[c_drop], perm[CK - 1] = perm[CK - 1], perm[c_drop]
        mk_hi, mk_lo = mk_hi[perm], mk_lo[perm]
        qk_hi, qk_lo = qk_hi[perm], qk_lo[perm]

        # [128, M]: rows 0:64 = hi, row 64 = ones, rows 65:128 = lo[0:63]
        mk_packed = np.empty((128, M), dtype=f16)
        mk_packed[0:CK] = mk_hi
        mk_packed[CK] = np.float16(1.0)
        mk_packed[CK + 1:] = mk_lo[0:CK - 1]

        rhsA_full = np.empty((128, N), dtype=f16)
        rhsA_full[0:CK] = qk_hi
        rhsA_full[CK] = np.float16(0.0)
        rhsA_full[CK + 1:] = qk_hi[0:CK - 1]

        rhsB_full = np.empty((128, N), dtype=f16)
        rhsB_full[0:CK] = qk_lo
        rhsB_full[CK] = np.float16(0.0)   # overwritten on device with -shift
        rhsB_full[CK + 1:] = qk_lo[0:CK - 1]

        mv = memory_values_low[b].transpose(0, 2, 3, 1).reshape(M, CV)
        mvp_full = np.empty((M, 257), dtype=f16)
        mvp_full[:, :256] = mv.astype(f16)
        mvp_full[:, 256] = np.float16(1.0)
        mvp_full = mvp_full.reshape(MT, 128, 257)

        for j in range(4):
            sl = slice(j * NCHUNK, (j + 1) * NCHUNK)
            qv_slice = np.ascontiguousarray(
                query_value[b][:, 16 * j:16 * (j + 1), :]).reshape(2, 128, 16, QW)
            in_maps.append({
                "mk": mk_packed,
                "rhsA": np.ascontiguousarray(rhsA_full[:, sl]),
                "rhsB": np.ascontiguousarray(rhsB_full[:, sl]),
                "mvp": mvp_full,
                "qv": qv_slice,
            })
    return in_maps


def _assemble(results):
    out = np.empty((B, 2 * CV, H, W), dtype=np.float32)
    for core, res in enumerate(results):
        b, j = divmod(core, 4)
        qvo = res["qvout"].reshape(CV, 8, 64)
        out[b, :CV, 8 * j:8 * (j + 1), :] = qvo
        mo = res["mout"].reshape(NCHUNK, CV).T  # [CV, 512]
        out[b, CV:, :, :].reshape(CV, N)[:, j * NCHUNK:(j + 1) * NCHUNK] = mo
    return out


def run(inputs, **kwargs):
    if "nc" not in _cached:
        _cached["nc"] = _build_program()
    nc = _cached["nc"]
    in_maps = _prep_inputs(
        np.asarray(inputs["query_value"], dtype=np.float32),
        np.asarray(inputs["memory_keys_low"], dtype=np.float32),
        np.asarray(inputs["memory_values_low"], dtype=np.float32),
        np.asarray(inputs["query_key_low"], dtype=np.float32),
    )
    res = run_bass_kernel_spmd(nc, in_maps, core_ids=list(range(NCORES)), **kwargs)
    return _assemble(res.results), res


def kernel(**inputs):
    out, _ = run(inputs)
    return out
